# revision 1
# baseline (speedup 1.0000x reference)
"""GCN (3-layer message passing + mean-pool + MLP head) on 8 Trainium2 NeuronCores.

Sharding: nodes and their incident (by dst) edges are sharded across 8 cores;
per layer the dinv-scaled features are AllGathered into a bf16 gather table in
DRAM, each core dma_gathers its edges' src rows and segment-sums them with
indicator matmuls on the TensorEngine (exact f32 PSUM accumulation, no
scatter-add races). Pooling is a per-core indicator matmul against batch ids
plus a tiny [129,256] AllReduce; the MLP head is computed redundantly.
"""

import numpy as np
from dataclasses import dataclass, field


# ---------------------------------------------------------------------------
# Config
# ---------------------------------------------------------------------------
@dataclass
class Cfg:
    N: int = 50000          # nodes
    E: int = 600000         # edges
    F: int = 128            # feature dim
    NL: int = 3             # gcn layers
    G: int = 256            # graphs
    H: int = 256            # hidden dim of head
    LD: int = 2             # label dim
    C: int = 8              # cores
    GCHUNK: int = 1024      # edges per dma_gather call
    IB: int = 8             # indicator subchunks built per DVE op

    @property
    def NPC(self):          # nodes per core
        return self.N // self.C

    @property
    def W(self):            # 128-node windows per core
        return (self.NPC + 127) // 128

    @property
    def NPAD(self):         # padded nodes per core
        return self.W * 128

    @property
    def TROWS(self):        # gather-table rows
        return self.C * self.NPAD

    @property
    def HA(self):           # local rows in table half A (window-aligned)
        return 128 * ((self.W + 1) // 2)

    @property
    def HB(self):           # local rows in table half B
        return self.NPAD - self.HA

    @property
    def SPLIT(self):        # low/high gather-stream boundary = half-A rows
        return self.C * self.HA


@dataclass
class Meta:
    """Uniform (core-independent) graph structure + per-core data arrays."""
    m_low: list = field(default_factory=list)    # per-window low subchunk count
    m_high: list = field(default_factory=list)   # per-window high subchunk count
    L_low: int = 0
    L_high: int = 0
    in_maps: list = field(default_factory=list)  # per-core tensor dicts


# ---------------------------------------------------------------------------
# Host-side sharding / layout prep (pure numpy, no model math)
# ---------------------------------------------------------------------------
def _wrap16(arr_i16):
    # slot i -> [i % 16, i // 16]; 16-row wrap replicated to 128 partitions
    # (one copy per GPSIMD Q7 core).
    return np.ascontiguousarray(np.tile(arr_i16.reshape(-1, 16).T, (8, 1)))


def _wrap128(arr_f32):
    # slot i -> [i % 128, i // 128]
    return np.ascontiguousarray(arr_f32.reshape(-1, 128).T)


def host_prep(cfg: Cfg, x, Wg, bg, w1, b1, w2, b2, edge_index, batch) -> Meta:
    C, NPC, W, NPAD = cfg.C, cfg.NPC, cfg.W, cfg.NPAD
    src = np.asarray(edge_index[0], dtype=np.int64)
    dst = np.asarray(edge_index[1], dtype=np.int64)
    batch = np.asarray(batch, dtype=np.int64)
    x = np.asarray(x, dtype=np.float32)

    # table row of a global node id: the table is two stacked AllGather
    # halves — rows [0, C*HA) hold every core's first HA local rows, rows
    # [C*HA, TROWS) the remaining HB (so each half is one collective).
    HA, HB = cfg.HA, cfg.HB
    sc, sl = src // NPC, src % NPC
    trow = np.where(sl < HA, sc * HA + sl, C * HA + sc * HB + (sl - HA))

    # per (core, window, half) edge lists
    per_core = []
    for c in range(C):
        m = (dst // NPC) == c
        s_c, d_c, t_c = src[m], dst[m], trow[m]
        dloc = d_c - c * NPC
        order = np.argsort(dloc, kind="stable")
        s_c, dloc, t_c = s_c[order], dloc[order], t_c[order]
        win = dloc // 128
        drel = dloc - win * 128
        lowm = t_c < cfg.SPLIT
        lists = []
        for w in range(W):
            wm = win == w
            lists.append((
                (t_c[wm & lowm], drel[wm & lowm]),
                (t_c[wm & ~lowm] - cfg.SPLIT, drel[wm & ~lowm]),
            ))
        per_core.append(lists)

    # uniform subchunk counts (max over cores), >=1 low subchunk per window
    m_low = [max(1, max(-(-len(per_core[c][w][0][0]) // 128) for c in range(C)))
             for w in range(W)]
    m_high = [max(-(-len(per_core[c][w][1][0]) // 128) for c in range(C))
              for w in range(W)]
    L_low = 128 * sum(m_low)
    L_high = 128 * sum(m_high)

    meta = Meta(m_low=m_low, m_high=m_high, L_low=L_low, L_high=L_high)

    gbase = 0
    for c in range(C):
        idx_low = np.zeros(L_low, np.int16)
        drel_low = np.full(L_low, -1.0, np.float32)
        idx_high = np.zeros(max(L_high, 128), np.int16)
        drel_high = np.full(max(L_high, 128), -1.0, np.float32)
        ol = oh = 0
        for w in range(W):
            (tl, dl), (th, dh) = per_core[c][w]
            idx_low[ol:ol + len(tl)] = tl.astype(np.int16)
            drel_low[ol:ol + len(dl)] = dl.astype(np.float32)
            ol += 128 * m_low[w]
            idx_high[oh:oh + len(th)] = th.astype(np.int16)
            drel_high[oh:oh + len(dh)] = dh.astype(np.float32)
            oh += 128 * m_high[w]
        assert ol == L_low and oh == L_high

        xs = np.zeros((NPAD, cfg.F), np.float32)
        xs[:NPC] = x[c * NPC:(c + 1) * NPC]

        batch_abs = np.full(NPAD, -1.0, np.float32)
        batch_abs[:NPC] = batch[c * NPC:(c + 1) * NPC].astype(np.float32)

        meta.in_maps.append(dict(
            xs=xs,
            src_low=_wrap16(idx_low),
            src_high=_wrap16(idx_high),
            drel_low=_wrap128(drel_low),
            drel_high=_wrap128(drel_high),
            batch_abs=_wrap128(batch_abs),
            Wg=np.asarray(Wg, np.float32),
            bg=np.asarray(bg, np.float32),
            w1=np.asarray(w1, np.float32),
            b1=np.asarray(b1, np.float32).reshape(cfg.H, 1),
            w2=np.asarray(w2, np.float32),
            b2=np.asarray(b2, np.float32).reshape(1, cfg.LD),
        ))
    return meta


# ---------------------------------------------------------------------------
# Device graph
# ---------------------------------------------------------------------------
def build_graph(cfg: Cfg, meta: Meta):
    import concourse.bass as bass
    import concourse.bacc as bacc
    import concourse.mybir as mybir
    import concourse.tile as tile

    F, W, NL, NPAD = cfg.F, cfg.W, cfg.NL, cfg.NPAD
    GR = cfg.G
    f32, bf16, i16 = mybir.dt.float32, mybir.dt.bfloat16, mybir.dt.int16
    AL = mybir.AluOpType
    ACT = mybir.ActivationFunctionType
    L_low, L_high = meta.L_low, meta.L_high
    LH_pad = max(L_high, 128)

    nc = bacc.Bacc("TRN2", target_bir_lowering=False, debug=False,
                   num_devices=cfg.C)

    # --- external IO ------------------------------------------------------
    P = {}
    P["xs"] = nc.declare_dram_parameter("xs", [NPAD, F], f32, isOutput=False)
    P["src_low"] = nc.declare_dram_parameter("src_low", [128, L_low // 16], i16, isOutput=False)
    P["src_high"] = nc.declare_dram_parameter("src_high", [128, LH_pad // 16], i16, isOutput=False)
    P["drel_low"] = nc.declare_dram_parameter("drel_low", [128, L_low // 128], f32, isOutput=False)
    P["drel_high"] = nc.declare_dram_parameter("drel_high", [128, LH_pad // 128], f32, isOutput=False)
    P["batch_abs"] = nc.declare_dram_parameter("batch_abs", [128, W], f32, isOutput=False)
    P["Wg"] = nc.declare_dram_parameter("Wg", [NL, F, F], f32, isOutput=False)
    P["bg"] = nc.declare_dram_parameter("bg", [NL, F], f32, isOutput=False)
    P["w1"] = nc.declare_dram_parameter("w1", [F, cfg.H], f32, isOutput=False)
    P["b1"] = nc.declare_dram_parameter("b1", [cfg.H, 1], f32, isOutput=False)
    P["w2"] = nc.declare_dram_parameter("w2", [cfg.H, cfg.LD], f32, isOutput=False)
    P["b2"] = nc.declare_dram_parameter("b2", [1, cfg.LD], f32, isOutput=False)
    out_ext = nc.declare_dram_parameter("out", [GR, cfg.LD], f32, isOutput=True)

    # --- internal DRAM ----------------------------------------------------
    tableDs = [nc.dram_tensor(f"tableD{i}", [cfg.TROWS, F], bf16,
                              addr_space="Shared") for i in range(2)]
    shardDs = [nc.dram_tensor(f"shardD{i}", [NPAD, F], bf16) for i in range(2)]
    degD = nc.dram_tensor("degD", [NPAD], f32)
    arInD = nc.dram_tensor("arInD", [129, GR], f32)
    arOutD = nc.dram_tensor("arOutD", [129, GR], f32, addr_space="Shared")

    rg = [list(range(cfg.C))]

    with tile.TileContext(nc) as tc:
        with (
            tc.tile_pool(name="res", bufs=1) as res,      # resident tensors
            tc.tile_pool(name="work", bufs=3) as work,    # per-window temps
            tc.tile_pool(name="indp", bufs=6) as indp,    # indicator batches
            tc.tile_pool(name="gbuf", bufs=6) as gpool,   # gather buffers
            tc.tile_pool(name="ps", bufs=2, space="PSUM") as ps,
            tc.tile_pool(name="ps3", bufs=3, space="PSUM") as ps3,
            tc.tile_pool(name="ps1", bufs=1, space="PSUM") as ps1,
            tc.tile_pool(name="psacc", bufs=1, space="PSUM") as psacc,
        ):
            # ---------------- resident loads / constants ----------------
            srcL = res.tile([128, L_low // 16], i16)
            nc.sync.dma_start(srcL[:], P["src_low"][:])
            srcH = res.tile([128, LH_pad // 16], i16)
            nc.sync.dma_start(srcH[:], P["src_high"][:])
            drelL = res.tile([128, L_low // 128], f32)
            nc.sync.dma_start(drelL[:], P["drel_low"][:])
            drelH = res.tile([128, LH_pad // 128], f32)
            nc.sync.dma_start(drelH[:], P["drel_high"][:])
            batchS = res.tile([128, W], f32)
            nc.sync.dma_start(batchS[:], P["batch_abs"][:])

            iotaF = res.tile([128, GR], f32)   # value = free index
            nc.gpsimd.iota(iotaF[:], pattern=[[1, GR]], base=0,
                           channel_multiplier=0,
                           allow_small_or_imprecise_dtypes=True)
            iotaC = res.tile([128, 1], f32)    # value = partition index
            nc.gpsimd.iota(iotaC[:], pattern=[[0, 1]], base=0,
                           channel_multiplier=1,
                           allow_small_or_imprecise_dtypes=True)
            ident = res.tile([128, 128], bf16)  # identity for PE transpose
            nc.vector.tensor_scalar(ident[:], iotaF[:, 0:128], iotaC[:], None,
                                    AL.is_equal)
            onesB = res.tile([128, 1], bf16)
            nc.vector.memset(onesB[:], 1.0)

            # weights -> bf16 SBUF
            wg_f = res.tile([128, NL * F], f32)
            for l in range(NL):
                nc.sync.dma_start(wg_f[:, l * F:(l + 1) * F], P["Wg"][l])
            wgS = res.tile([128, NL * F], bf16)
            nc.vector.tensor_copy(wgS[:], wg_f[:])

            bg_row = res.tile([1, NL * F], f32)
            nc.sync.dma_start(bg_row[:], P["bg"][:].rearrange("l f -> (l f)"))
            bgB = res.tile([128, NL * F], f32)
            nc.gpsimd.partition_broadcast(bgB[:], bg_row[:])

            w1S = res.tile([128, cfg.H], f32)
            nc.sync.dma_start(w1S[:], P["w1"][:])
            w2S = res.tile([128, 2 * cfg.LD], f32)
            nc.sync.dma_start(w2S[:, 0:cfg.LD], P["w2"][0:128, :])
            nc.sync.dma_start(w2S[:, cfg.LD:2 * cfg.LD], P["w2"][128:256, :])

            b1S = res.tile([128, 2], f32)
            nc.sync.dma_start(b1S[:, 0:1], P["b1"][0:128, :])
            nc.sync.dma_start(b1S[:, 1:2], P["b1"][128:256, :])
            b2_row = res.tile([1, cfg.LD], f32)
            nc.sync.dma_start(b2_row[:], P["b2"][:])
            b2B = res.tile([128, cfg.LD], f32)
            nc.gpsimd.partition_broadcast(b2B[:], b2_row[:])

            hbuf = res.tile([128, W * F], bf16)      # h' chunks, node-major
            degRow = res.tile([1, W * 128], f32)
            dinvS = res.tile([128, W], f32)

            # subchunk -> (stream, col, window) schedule, window-major
            sched = []   # per window: list of (drel_tile, col, src_is_low)
            colL = colH = 0
            for w in range(W):
                lst = []
                for _ in range(meta.m_low[w]):
                    lst.append(("L", colL))
                    colL += 1
                for _ in range(meta.m_high[w]):
                    lst.append(("H", colH))
                    colH += 1
                sched.append(lst)

            # batched indicator builder: one DVE op builds IB subchunks'
            # [128e x 128n] one-hot tiles side by side (dstrel values are
            # window-relative, so the iota comparison is subchunk-invariant)
            IB = cfg.IB
            nsubL = L_low // 128
            nsubH = LH_pad // 128
            iotaT = res.tile([128, IB * 128], f32)
            nc.gpsimd.iota(iotaT[:], pattern=[[0, IB], [1, 128]], base=0,
                           channel_multiplier=0,
                           allow_small_or_imprecise_dtypes=True)

            def make_ind_getter():
                cache = {}

                def get(stream, col):
                    drel, nsub = (drelL, nsubL) if stream == "L" else (drelH, nsubH)
                    s0 = col - col % IB
                    key = (stream, s0)
                    if key not in cache:
                        nb = min(IB, nsub - s0)
                        it = indp.tile([128, IB * 128], bf16, tag="ind")
                        dsl = drel[:, s0:s0 + nb]
                        din = bass.AP(dsl.tensor, dsl.offset,
                                      [list(d) for d in dsl.ap] + [[0, 128]])
                        nc.vector.tensor_tensor(
                            it[:].rearrange("p (c e) -> p c e", e=128)[:, 0:nb, :],
                            din,
                            iotaT[:].rearrange("p (c e) -> p c e", e=128)[:, 0:nb, :],
                            AL.is_equal)
                        cache[key] = it
                        for k in [k for k in cache
                                  if k[0] == stream and k[1] < s0 - IB]:
                            del cache[k]
                    return (cache[key][:]
                            .rearrange("p (c e) -> p c e", e=128)[:, col - s0, :])
                return get

            # gather-call layout per stream
            def gather_calls(L_tot):
                calls = []
                s = 0
                while s < L_tot:
                    n = min(cfg.GCHUNK, L_tot - s)
                    calls.append((s, n))
                    s += n
                return calls
            callsL = gather_calls(L_low)
            callsH = gather_calls(L_high)

            HA, WA = cfg.HA, cfg.HA // 128

            def emit_shard_half(half, par):
                shardD, tableD = shardDs[par], tableDs[par]
                r0, r1 = (0, HA) if half == 0 else (HA, NPAD)
                t0 = 0 if half == 0 else cfg.C * HA
                nc.sync.dma_start(
                    shardD[r0:r1].rearrange("(w p) f -> p w f", p=128),
                    hbuf[:, r0 * F // 128:r1 * F // 128]
                    .rearrange("p (w f) -> p w f", f=F))
                nc.gpsimd.collective_compute(
                    "AllGather", mybir.AluOpType.bypass, replica_groups=rg,
                    ins=[shardD[r0:r1]],
                    outs=[tableD[t0:t0 + cfg.C * (r1 - r0)]])

            # per-layer gather-call plan, ordered by first consuming window so
            # calls can be emitted just-in-time inside the window loop
            call_plan = []   # (w_start, stream, s0, n)
            for stream, calls, m in (("L", callsL, meta.m_low),
                                     ("H", callsH, meta.m_high)):
                win_of_col = [w for w in range(W) for _ in range(m[w])]
                for (s0, n) in calls:
                    call_plan.append((win_of_col[s0 // 128], stream, s0, n))
            call_plan.sort(key=lambda t: t[0])

            # ------- deg pass + layer-0 phase A, in halves so the first
            # table half's collective overlaps the second half's compute ----
            degCol = res.tile([128, W], f32)
            get_ind = make_ind_getter()
            for half, (w0, w1) in enumerate(((0, WA), (WA, W))):
                for w in range(w0, w1):
                    degP = psacc.tile([1, 128], f32, tag="acc1")
                    n = len(sched[w])
                    for i, (stream, col) in enumerate(sched[w]):
                        nc.tensor.matmul(degP[:], onesB[:], get_ind(stream, col),
                                         start=(i == 0), stop=(i == n - 1))
                    nc.vector.tensor_copy(degRow[:, w * 128:(w + 1) * 128],
                                          degP[:])
                nc.sync.dma_start(
                    degD[w0 * 128:w1 * 128].rearrange("(a b) -> a b", a=1),
                    degRow[:, w0 * 128:w1 * 128])
                nc.sync.dma_start(
                    degCol[:, w0:w1],
                    degD[w0 * 128:w1 * 128].rearrange("(w p) -> p w", p=128))
                sq = work.tile([128, w1 - w0], f32, tag="sq")
                nc.scalar.activation(sq[:], degCol[:, w0:w1], ACT.Sqrt, bias=1.0)
                nc.vector.reciprocal(dinvS[:, w0:w1], sq[:])
                # layer-0 phase A for this half: h' = dinv * (x0 @ Wg0)
                for w in range(w0, w1):
                    xcf = work.tile([128, F], f32, tag="xcf")
                    nc.sync.dma_start(xcf[:], P["xs"][w * 128:(w + 1) * 128, :])
                    xcb = work.tile([128, F], bf16, tag="xcb")
                    nc.vector.tensor_copy(xcb[:], xcf[:])
                    xtP = ps1.tile([128, F], bf16, tag="xtP")
                    nc.tensor.transpose(xtP[:], xcb[:], ident[:])
                    xt = work.tile([128, F], bf16, tag="xt")
                    nc.vector.tensor_copy(xt[:], xtP[:])
                    hP = ps.tile([128, F], f32, tag="hP")
                    nc.tensor.matmul(hP[:], xt[:], wgS[:, 0:F],
                                     start=True, stop=True)
                    nc.vector.tensor_scalar(hbuf[:, w * F:(w + 1) * F], hP[:],
                                            dinvS[:, w:w + 1], None, AL.mult)
                emit_shard_half(half, 0)

            poolP = None
            cntP = None
            for l in range(NL):
                gtiles = {}

                tableD = tableDs[l % 2]

                def emit_call(stream, s0, n):
                    srcT, base, rows = ((srcL, 0, cfg.SPLIT) if stream == "L"
                                        else (srcH, cfg.SPLIT,
                                              cfg.TROWS - cfg.SPLIT))
                    gt = gpool.tile([128, (n // 128) * F], bf16,
                                    tag=f"g{stream}")
                    nc.gpsimd.dma_gather(
                        gt[:].rearrange("p (c e) -> p c e", e=F),
                        tableD[base:base + rows, :],
                        srcT[:, s0 // 16:(s0 + n) // 16],
                        n, n, F)
                    gtiles[(stream, s0)] = gt

                def gslice(stream, col):
                    calls = callsL if stream == "L" else callsH
                    for (s0, n) in calls:
                        if s0 <= col * 128 < s0 + n:
                            gt = gtiles[(stream, s0)]
                            j = col - s0 // 128
                            return gt[:].rearrange("p (c e) -> p c e", e=F)[:, j, :]
                    raise AssertionError

                last = l == NL - 1
                if last:
                    poolP = psacc.tile([128, GR], f32, tag="poolP")
                    cntP = psacc.tile([1, GR], f32, tag="acc1")

                get_ind = make_ind_getter()
                ci = 0
                for w in range(W):
                    while ci < len(call_plan) and call_plan[ci][0] <= w:
                        emit_call(*call_plan[ci][1:])
                        ci += 1
                    SP = ps3.tile([128, F], f32, tag="SP")
                    n = len(sched[w])
                    for i, (stream, col) in enumerate(sched[w]):
                        nc.tensor.matmul(SP[:], get_ind(stream, col),
                                         gslice(stream, col),
                                         start=(i == 0), stop=(i == n - 1))
                    # x_new = relu(dinv*SP + bg + dinv*h')
                    u = work.tile([128, F], f32, tag="u")
                    nc.vector.scalar_tensor_tensor(
                        u[:], SP[:], dinvS[:, w:w + 1],
                        bgB[:, l * F:(l + 1) * F], AL.mult, AL.add)
                    v = work.tile([128, F], f32, tag="v")
                    nc.vector.scalar_tensor_tensor(
                        v[:], hbuf[:, w * F:(w + 1) * F], dinvS[:, w:w + 1],
                        u[:], AL.mult, AL.add)
                    xn = work.tile([128, F], bf16, tag="xn")
                    nc.scalar.activation(xn[:], v[:], ACT.Relu)

                    if not last:
                        xtP = ps1.tile([128, F], bf16, tag="xtP")
                        nc.tensor.transpose(xtP[:], xn[:], ident[:])
                        xt = work.tile([128, F], bf16, tag="xt")
                        nc.vector.tensor_copy(xt[:], xtP[:])
                        hP = ps.tile([128, F], f32, tag="hP")
                        nc.tensor.matmul(hP[:], xt[:],
                                         wgS[:, (l + 1) * F:(l + 2) * F],
                                         start=True, stop=True)
                        nc.vector.tensor_scalar(
                            hbuf[:, w * F:(w + 1) * F], hP[:],
                            dinvS[:, w:w + 1], None, AL.mult)
                        if w == WA - 1:
                            emit_shard_half(0, (l + 1) % 2)
                    else:
                        ig = work.tile([128, GR], bf16, tag="ig")
                        nc.vector.tensor_scalar(ig[:], iotaF[:],
                                                batchS[:, w:w + 1], None,
                                                AL.is_equal)
                        nc.tensor.matmul(poolP[:], xn[:], ig[:],
                                         start=(w == 0), stop=(w == W - 1),
                                         skip_group_check=True)
                        nc.tensor.matmul(cntP[:], onesB[:], ig[:],
                                         start=(w == 0), stop=(w == W - 1),
                                         skip_group_check=True)
                assert ci == len(call_plan)
                if not last:
                    emit_shard_half(1, (l + 1) % 2)

            # ---------------- pooling allreduce + head ----------------
            sumsS = work.tile([128, GR], f32, tag="sumsS")
            nc.vector.tensor_copy(sumsS[:], poolP[:])
            cntS = work.tile([1, GR], f32, tag="cntS")
            nc.vector.tensor_copy(cntS[:], cntP[:])
            nc.sync.dma_start(arInD[0:128, :], sumsS[:])
            nc.sync.dma_start(arInD[128:129, :], cntS[:])
            nc.gpsimd.collective_compute(
                "AllReduce", mybir.AluOpType.add, replica_groups=rg,
                ins=[arInD[:]], outs=[arOutD[:]])
            sumsA = work.tile([128, GR], f32, tag="sumsA")
            nc.sync.dma_start(sumsA[:], arOutD[0:128, :])
            cntA = work.tile([1, GR], f32, tag="cntA")
            nc.sync.dma_start(cntA[:], arOutD[128:129, :])
            cntM = work.tile([1, GR], f32, tag="cntM")
            nc.vector.tensor_scalar(cntM[:], cntA[:], 1.0, None, AL.max)
            rec = work.tile([1, GR], f32, tag="rec")
            nc.vector.reciprocal(rec[:], cntM[:])
            recB = work.tile([128, GR], f32, tag="recB")
            nc.gpsimd.partition_broadcast(recB[:], rec[:])
            pooledT = work.tile([128, GR], f32, tag="pooledT")
            nc.vector.tensor_tensor(pooledT[:], sumsA[:], recB[:], AL.mult)

            h1 = []
            for h in range(2):
                h1P = ps3.tile([128, GR], f32, tag="SP")
                nc.tensor.matmul(h1P[:], w1S[:, h * 128:(h + 1) * 128],
                                 pooledT[:], start=True, stop=True)
                h1S = work.tile([128, GR], f32, tag=f"h1S{h}")
                nc.scalar.activation(h1S[:], h1P[:], ACT.Relu,
                                     bias=b1S[:, h:h + 1])
                h1.append(h1S)
            for g in range(GR // 128):
                oP = ps3.tile([128, cfg.LD], f32, tag="SP")
                nc.tensor.matmul(oP[:], h1[0][:, g * 128:(g + 1) * 128],
                                 w2S[:, 0:cfg.LD], start=True, stop=False)
                nc.tensor.matmul(oP[:], h1[1][:, g * 128:(g + 1) * 128],
                                 w2S[:, cfg.LD:2 * cfg.LD], start=False, stop=True)
                oS = work.tile([128, cfg.LD], f32, tag="oS")
                nc.vector.tensor_tensor(oS[:], oP[:], b2B[:], AL.add)
                nc.sync.dma_start(out_ext[g * 128:(g + 1) * 128, :], oS[:])

    nc.compile()
    return nc


# ---------------------------------------------------------------------------
# Entry point
# ---------------------------------------------------------------------------
_CACHE = {}


def _build(cfg, meta):
    key = (tuple(meta.m_low), tuple(meta.m_high))
    if key not in _CACHE:
        _CACHE[key] = build_graph(cfg, meta)
    return _CACHE[key]


def kernel(**inputs) -> np.ndarray:
    from concourse.bass_utils import run_bass_kernel_spmd
    cfg = Cfg()
    meta = host_prep(cfg, **inputs)
    nc = _build(cfg, meta)
    res = run_bass_kernel_spmd(nc, meta.in_maps, list(range(cfg.C)))
    return np.asarray(res.results[0]["out"], dtype=np.float32)



# revision 3
# speedup vs baseline: 2.1092x; 2.1092x over previous
"""GCN (3-layer message passing + mean-pool + MLP head) on 8 Trainium2 NeuronCores.

Sharding: nodes and their incident (by dst) edges are sharded across 8 cores;
per layer the dinv-scaled features are AllGathered into a bf16 gather table in
DRAM, each core dma_gathers its edges' src rows and segment-sums them with
indicator matmuls on the TensorEngine (exact f32 PSUM accumulation, no
scatter-add races). Pooling is a per-core indicator matmul against batch ids
plus a tiny [129,256] AllReduce; the MLP head is computed redundantly.
"""

import numpy as np
from dataclasses import dataclass, field


# ---------------------------------------------------------------------------
# Config
# ---------------------------------------------------------------------------
@dataclass
class Cfg:
    N: int = 50000          # nodes
    E: int = 600000         # edges
    F: int = 128            # feature dim
    NL: int = 3             # gcn layers
    G: int = 256            # graphs
    H: int = 256            # hidden dim of head
    LD: int = 2             # label dim
    C: int = 8              # cores
    GCHUNK: int = 1024      # edges per dma_gather call
    IB: int = 8             # indicator subchunks built per DVE op

    @property
    def NPC(self):          # nodes per core
        return self.N // self.C

    @property
    def W(self):            # 128-node windows per core
        return (self.NPC + 127) // 128

    @property
    def NPAD(self):         # padded nodes per core
        return self.W * 128

    @property
    def TROWS(self):        # gather-table rows
        return self.C * self.NPAD

    @property
    def HA(self):           # local rows in table half A (window-aligned)
        return 128 * ((self.W + 1) // 2)

    @property
    def HB(self):           # local rows in table half B
        return self.NPAD - self.HA

    @property
    def SPLIT(self):        # low/high gather-stream boundary = half-A rows
        return self.C * self.HA


@dataclass
class Meta:
    """Uniform (core-independent) graph structure + per-core data arrays."""
    m_low: list = field(default_factory=list)    # per-window low subchunk count
    m_high: list = field(default_factory=list)   # per-window high subchunk count
    L_low: int = 0
    L_high: int = 0
    in_maps: list = field(default_factory=list)  # per-core tensor dicts


# ---------------------------------------------------------------------------
# Host-side sharding / layout prep (pure numpy, no model math)
# ---------------------------------------------------------------------------
def _wrap16(arr_i16):
    # slot i -> [i % 16, i // 16]; 16-row wrap replicated to 128 partitions
    # (one copy per GPSIMD Q7 core).
    return np.ascontiguousarray(np.tile(arr_i16.reshape(-1, 16).T, (8, 1)))


def _wrap128(arr_f32):
    # slot i -> [i % 128, i // 128]
    return np.ascontiguousarray(arr_f32.reshape(-1, 128).T)


def host_prep(cfg: Cfg, x, Wg, bg, w1, b1, w2, b2, edge_index, batch) -> Meta:
    C, NPC, W, NPAD = cfg.C, cfg.NPC, cfg.W, cfg.NPAD
    src = np.asarray(edge_index[0], dtype=np.int64)
    dst = np.asarray(edge_index[1], dtype=np.int64)
    batch = np.asarray(batch, dtype=np.int64)
    x = np.asarray(x, dtype=np.float32)

    # table row of a global node id: the table is two stacked AllGather
    # halves — rows [0, C*HA) hold every core's first HA local rows, rows
    # [C*HA, TROWS) the remaining HB (so each half is one collective).
    HA, HB = cfg.HA, cfg.HB
    sc, sl = src // NPC, src % NPC
    trow = np.where(sl < HA, sc * HA + sl, C * HA + sc * HB + (sl - HA))

    # per (core, window, half) edge lists
    per_core = []
    for c in range(C):
        m = (dst // NPC) == c
        s_c, d_c, t_c = src[m], dst[m], trow[m]
        dloc = d_c - c * NPC
        order = np.argsort(dloc, kind="stable")
        s_c, dloc, t_c = s_c[order], dloc[order], t_c[order]
        win = dloc // 128
        drel = dloc - win * 128
        lowm = t_c < cfg.SPLIT
        lists = []
        for w in range(W):
            wm = win == w
            lists.append((
                (t_c[wm & lowm], drel[wm & lowm]),
                (t_c[wm & ~lowm] - cfg.SPLIT, drel[wm & ~lowm]),
            ))
        per_core.append(lists)

    # uniform subchunk counts (max over cores), >=1 low subchunk per window
    m_low = [max(1, max(-(-len(per_core[c][w][0][0]) // 128) for c in range(C)))
             for w in range(W)]
    m_high = [max(-(-len(per_core[c][w][1][0]) // 128) for c in range(C))
              for w in range(W)]
    L_low = 128 * sum(m_low)
    L_high = 128 * sum(m_high)

    meta = Meta(m_low=m_low, m_high=m_high, L_low=L_low, L_high=L_high)

    gbase = 0
    for c in range(C):
        idx_low = np.zeros(L_low, np.int16)
        drel_low = np.full(L_low, -1.0, np.float32)
        idx_high = np.zeros(max(L_high, 128), np.int16)
        drel_high = np.full(max(L_high, 128), -1.0, np.float32)
        ol = oh = 0
        for w in range(W):
            (tl, dl), (th, dh) = per_core[c][w]
            idx_low[ol:ol + len(tl)] = tl.astype(np.int16)
            drel_low[ol:ol + len(dl)] = dl.astype(np.float32)
            ol += 128 * m_low[w]
            idx_high[oh:oh + len(th)] = th.astype(np.int16)
            drel_high[oh:oh + len(dh)] = dh.astype(np.float32)
            oh += 128 * m_high[w]
        assert ol == L_low and oh == L_high

        xs = np.zeros((NPAD, cfg.F), np.float32)
        xs[:NPC] = x[c * NPC:(c + 1) * NPC]

        batch_abs = np.full(NPAD, -1.0, np.float32)
        batch_abs[:NPC] = batch[c * NPC:(c + 1) * NPC].astype(np.float32)

        meta.in_maps.append(dict(
            xs=xs,
            src_low=_wrap16(idx_low),
            src_high=_wrap16(idx_high),
            drel_low=_wrap128(drel_low),
            drel_high=_wrap128(drel_high),
            batch_abs=_wrap128(batch_abs),
            Wg=np.asarray(Wg, np.float32),
            bg=np.asarray(bg, np.float32),
            w1=np.asarray(w1, np.float32),
            b1=np.asarray(b1, np.float32).reshape(cfg.H, 1),
            w2=np.asarray(w2, np.float32),
            b2=np.asarray(b2, np.float32).reshape(1, cfg.LD),
        ))
    return meta


# ---------------------------------------------------------------------------
# Device graph
# ---------------------------------------------------------------------------
def build_graph(cfg: Cfg, meta: Meta):
    import concourse.bass as bass
    import concourse.bacc as bacc
    import concourse.mybir as mybir
    import concourse.tile as tile

    F, W, NL, NPAD = cfg.F, cfg.W, cfg.NL, cfg.NPAD
    GR = cfg.G
    f32, bf16, i16 = mybir.dt.float32, mybir.dt.bfloat16, mybir.dt.int16
    AL = mybir.AluOpType
    ACT = mybir.ActivationFunctionType
    L_low, L_high = meta.L_low, meta.L_high
    LH_pad = max(L_high, 128)

    nc = bacc.Bacc("TRN2", target_bir_lowering=False, debug=False,
                   num_devices=cfg.C, num_swdge_queues=4)

    # --- external IO ------------------------------------------------------
    P = {}
    P["xs"] = nc.declare_dram_parameter("xs", [NPAD, F], f32, isOutput=False)
    P["src_low"] = nc.declare_dram_parameter("src_low", [128, L_low // 16], i16, isOutput=False)
    P["src_high"] = nc.declare_dram_parameter("src_high", [128, LH_pad // 16], i16, isOutput=False)
    P["drel_low"] = nc.declare_dram_parameter("drel_low", [128, L_low // 128], f32, isOutput=False)
    P["drel_high"] = nc.declare_dram_parameter("drel_high", [128, LH_pad // 128], f32, isOutput=False)
    P["batch_abs"] = nc.declare_dram_parameter("batch_abs", [128, W], f32, isOutput=False)
    P["Wg"] = nc.declare_dram_parameter("Wg", [NL, F, F], f32, isOutput=False)
    P["bg"] = nc.declare_dram_parameter("bg", [NL, F], f32, isOutput=False)
    P["w1"] = nc.declare_dram_parameter("w1", [F, cfg.H], f32, isOutput=False)
    P["b1"] = nc.declare_dram_parameter("b1", [cfg.H, 1], f32, isOutput=False)
    P["w2"] = nc.declare_dram_parameter("w2", [cfg.H, cfg.LD], f32, isOutput=False)
    P["b2"] = nc.declare_dram_parameter("b2", [1, cfg.LD], f32, isOutput=False)
    out_ext = nc.declare_dram_parameter("out", [GR, cfg.LD], f32, isOutput=True)

    # --- internal DRAM ----------------------------------------------------
    tableDs = [nc.dram_tensor(f"tableD{i}", [cfg.TROWS, F], bf16,
                              addr_space="Shared") for i in range(2)]
    shardDs = [nc.dram_tensor(f"shardD{i}", [NPAD, F], bf16) for i in range(2)]
    degD = nc.dram_tensor("degD", [NPAD], f32)
    arInD = nc.dram_tensor("arInD", [129, GR], f32)
    arOutD = nc.dram_tensor("arOutD", [129, GR], f32, addr_space="Shared")

    rg = [list(range(cfg.C))]

    with tile.TileContext(nc) as tc:
        with (
            tc.tile_pool(name="res", bufs=1) as res,      # resident tensors
            tc.tile_pool(name="work", bufs=3) as work,    # per-window temps
            tc.tile_pool(name="indp", bufs=6) as indp,    # indicator batches
            tc.tile_pool(name="gbuf", bufs=6) as gpool,   # gather buffers
            tc.tile_pool(name="ps", bufs=2, space="PSUM") as ps,
            tc.tile_pool(name="ps3", bufs=3, space="PSUM") as ps3,
            tc.tile_pool(name="ps1", bufs=1, space="PSUM") as ps1,
            tc.tile_pool(name="psacc", bufs=1, space="PSUM") as psacc,
        ):
            # ---------------- resident loads / constants ----------------
            srcL = res.tile([128, L_low // 16], i16)
            nc.sync.dma_start(srcL[:], P["src_low"][:])
            srcH = res.tile([128, LH_pad // 16], i16)
            nc.sync.dma_start(srcH[:], P["src_high"][:])
            drelL = res.tile([128, L_low // 128], f32)
            nc.sync.dma_start(drelL[:], P["drel_low"][:])
            drelH = res.tile([128, LH_pad // 128], f32)
            nc.sync.dma_start(drelH[:], P["drel_high"][:])
            batchS = res.tile([128, W], f32)
            nc.sync.dma_start(batchS[:], P["batch_abs"][:])

            iotaF = res.tile([128, GR], f32)   # value = free index
            nc.gpsimd.iota(iotaF[:], pattern=[[1, GR]], base=0,
                           channel_multiplier=0,
                           allow_small_or_imprecise_dtypes=True)
            iotaC = res.tile([128, 1], f32)    # value = partition index
            nc.gpsimd.iota(iotaC[:], pattern=[[0, 1]], base=0,
                           channel_multiplier=1,
                           allow_small_or_imprecise_dtypes=True)
            ident = res.tile([128, 128], bf16)  # identity for PE transpose
            nc.vector.tensor_scalar(ident[:], iotaF[:, 0:128], iotaC[:], None,
                                    AL.is_equal)
            onesB = res.tile([128, 1], bf16)
            nc.vector.memset(onesB[:], 1.0)

            # weights -> bf16 SBUF
            wg_f = res.tile([128, NL * F], f32)
            for l in range(NL):
                nc.sync.dma_start(wg_f[:, l * F:(l + 1) * F], P["Wg"][l])
            wgS = res.tile([128, NL * F], bf16)
            nc.vector.tensor_copy(wgS[:], wg_f[:])

            bg_row = res.tile([1, NL * F], f32)
            nc.sync.dma_start(bg_row[:], P["bg"][:].rearrange("l f -> (l f)"))
            bgB = res.tile([128, NL * F], f32)
            nc.gpsimd.partition_broadcast(bgB[:], bg_row[:])

            w1S = res.tile([128, cfg.H], f32)
            nc.sync.dma_start(w1S[:], P["w1"][:])
            w2S = res.tile([128, 2 * cfg.LD], f32)
            nc.sync.dma_start(w2S[:, 0:cfg.LD], P["w2"][0:128, :])
            nc.sync.dma_start(w2S[:, cfg.LD:2 * cfg.LD], P["w2"][128:256, :])

            b1S = res.tile([128, 2], f32)
            nc.sync.dma_start(b1S[:, 0:1], P["b1"][0:128, :])
            nc.sync.dma_start(b1S[:, 1:2], P["b1"][128:256, :])
            b2_row = res.tile([1, cfg.LD], f32)
            nc.sync.dma_start(b2_row[:], P["b2"][:])
            b2B = res.tile([128, cfg.LD], f32)
            nc.gpsimd.partition_broadcast(b2B[:], b2_row[:])

            hbuf = res.tile([128, W * F], bf16)      # h' chunks, node-major
            degRow = res.tile([1, W * 128], f32)
            dinvS = res.tile([128, W], f32)

            # subchunk -> (stream, col, window) schedule, window-major
            sched = []   # per window: list of (drel_tile, col, src_is_low)
            colL = colH = 0
            for w in range(W):
                lst = []
                for _ in range(meta.m_low[w]):
                    lst.append(("L", colL))
                    colL += 1
                for _ in range(meta.m_high[w]):
                    lst.append(("H", colH))
                    colH += 1
                sched.append(lst)

            # batched indicator builder: one DVE op builds IB subchunks'
            # [128e x 128n] one-hot tiles side by side (dstrel values are
            # window-relative, so the iota comparison is subchunk-invariant)
            IB = cfg.IB
            nsubL = L_low // 128
            nsubH = LH_pad // 128
            iotaT = res.tile([128, IB * 128], f32)
            nc.gpsimd.iota(iotaT[:], pattern=[[0, IB], [1, 128]], base=0,
                           channel_multiplier=0,
                           allow_small_or_imprecise_dtypes=True)

            def make_ind_getter():
                cache = {}

                def get(stream, col):
                    drel, nsub = (drelL, nsubL) if stream == "L" else (drelH, nsubH)
                    s0 = col - col % IB
                    key = (stream, s0)
                    if key not in cache:
                        nb = min(IB, nsub - s0)
                        it = indp.tile([128, IB * 128], bf16, tag="ind")
                        dsl = drel[:, s0:s0 + nb]
                        din = bass.AP(dsl.tensor, dsl.offset,
                                      [list(d) for d in dsl.ap] + [[0, 128]])
                        nc.vector.tensor_tensor(
                            it[:].rearrange("p (c e) -> p c e", e=128)[:, 0:nb, :],
                            din,
                            iotaT[:].rearrange("p (c e) -> p c e", e=128)[:, 0:nb, :],
                            AL.is_equal)
                        cache[key] = it
                        for k in [k for k in cache
                                  if k[0] == stream and k[1] < s0 - IB]:
                            del cache[k]
                    return (cache[key][:]
                            .rearrange("p (c e) -> p c e", e=128)[:, col - s0, :])
                return get

            # gather-call layout per stream
            def gather_calls(L_tot):
                calls = []
                s = 0
                while s < L_tot:
                    n = min(cfg.GCHUNK, L_tot - s)
                    calls.append((s, n))
                    s += n
                return calls
            callsL = gather_calls(L_low)
            callsH = gather_calls(L_high)

            HA, WA = cfg.HA, cfg.HA // 128

            def emit_shard_half(half, par):
                shardD, tableD = shardDs[par], tableDs[par]
                r0, r1 = (0, HA) if half == 0 else (HA, NPAD)
                t0 = 0 if half == 0 else cfg.C * HA
                nc.sync.dma_start(
                    shardD[r0:r1].rearrange("(w p) f -> p w f", p=128),
                    hbuf[:, r0 * F // 128:r1 * F // 128]
                    .rearrange("p (w f) -> p w f", f=F))
                nc.gpsimd.collective_compute(
                    "AllGather", mybir.AluOpType.bypass, replica_groups=rg,
                    ins=[shardD[r0:r1]],
                    outs=[tableD[t0:t0 + cfg.C * (r1 - r0)]])

            # per-layer gather-call plan, ordered by first consuming window so
            # calls can be emitted just-in-time inside the window loop
            call_plan = []   # (w_start, stream, s0, n)
            for stream, calls, m in (("L", callsL, meta.m_low),
                                     ("H", callsH, meta.m_high)):
                win_of_col = [w for w in range(W) for _ in range(m[w])]
                for (s0, n) in calls:
                    call_plan.append((win_of_col[s0 // 128], stream, s0, n))
            call_plan.sort(key=lambda t: t[0])

            # ------- deg pass + layer-0 phase A, in halves so the first
            # table half's collective overlaps the second half's compute ----
            degCol = res.tile([128, W], f32)
            get_ind = make_ind_getter()
            for half, (w0, w1) in enumerate(((0, WA), (WA, W))):
                for w in range(w0, w1):
                    degP = psacc.tile([1, 128], f32, tag="acc1")
                    n = len(sched[w])
                    for i, (stream, col) in enumerate(sched[w]):
                        nc.tensor.matmul(degP[:], onesB[:], get_ind(stream, col),
                                         start=(i == 0), stop=(i == n - 1))
                    nc.vector.tensor_copy(degRow[:, w * 128:(w + 1) * 128],
                                          degP[:])
                nc.sync.dma_start(
                    degD[w0 * 128:w1 * 128].rearrange("(a b) -> a b", a=1),
                    degRow[:, w0 * 128:w1 * 128])
                nc.sync.dma_start(
                    degCol[:, w0:w1],
                    degD[w0 * 128:w1 * 128].rearrange("(w p) -> p w", p=128))
                sq = work.tile([128, w1 - w0], f32, tag="sq")
                nc.scalar.activation(sq[:], degCol[:, w0:w1], ACT.Sqrt, bias=1.0)
                nc.vector.reciprocal(dinvS[:, w0:w1], sq[:])
                # layer-0 phase A for this half: h' = dinv * (x0 @ Wg0)
                for w in range(w0, w1):
                    xcf = work.tile([128, F], f32, tag="xcf")
                    nc.sync.dma_start(xcf[:], P["xs"][w * 128:(w + 1) * 128, :])
                    xcb = work.tile([128, F], bf16, tag="xcb")
                    nc.vector.tensor_copy(xcb[:], xcf[:])
                    xtP = ps1.tile([128, F], bf16, tag="xtP")
                    nc.tensor.transpose(xtP[:], xcb[:], ident[:])
                    xt = work.tile([128, F], bf16, tag="xt")
                    nc.vector.tensor_copy(xt[:], xtP[:])
                    hP = ps.tile([128, F], f32, tag="hP")
                    nc.tensor.matmul(hP[:], xt[:], wgS[:, 0:F],
                                     start=True, stop=True)
                    nc.vector.tensor_scalar(hbuf[:, w * F:(w + 1) * F], hP[:],
                                            dinvS[:, w:w + 1], None, AL.mult)
                emit_shard_half(half, 0)

            poolP = None
            cntP = None
            for l in range(NL):
                gtiles = {}
                qctr = [0]

                tableD = tableDs[l % 2]

                def emit_call(stream, s0, n):
                    srcT, base, rows = ((srcL, 0, cfg.SPLIT) if stream == "L"
                                        else (srcH, cfg.SPLIT,
                                              cfg.TROWS - cfg.SPLIT))
                    gt = gpool.tile([128, (n // 128) * F], bf16,
                                    tag=f"g{stream}")
                    nc.gpsimd.dma_gather(
                        gt[:].rearrange("p (c e) -> p c e", e=F),
                        tableD[base:base + rows, :],
                        srcT[:, s0 // 16:(s0 + n) // 16],
                        n, n, F,
                        queue_num=qctr[0] % 4)
                    qctr[0] += 1
                    gtiles[(stream, s0)] = gt

                def gslice(stream, col):
                    calls = callsL if stream == "L" else callsH
                    for (s0, n) in calls:
                        if s0 <= col * 128 < s0 + n:
                            gt = gtiles[(stream, s0)]
                            j = col - s0 // 128
                            return gt[:].rearrange("p (c e) -> p c e", e=F)[:, j, :]
                    raise AssertionError

                last = l == NL - 1
                if last:
                    poolP = psacc.tile([128, GR], f32, tag="poolP")
                    cntP = psacc.tile([1, GR], f32, tag="acc1")

                get_ind = make_ind_getter()
                ci = 0
                for w in range(W):
                    while ci < len(call_plan) and call_plan[ci][0] <= w:
                        emit_call(*call_plan[ci][1:])
                        ci += 1
                    SP = ps3.tile([128, F], f32, tag="SP")
                    n = len(sched[w])
                    for i, (stream, col) in enumerate(sched[w]):
                        nc.tensor.matmul(SP[:], get_ind(stream, col),
                                         gslice(stream, col),
                                         start=(i == 0), stop=(i == n - 1))
                    # x_new = relu(dinv*SP + bg + dinv*h')
                    u = work.tile([128, F], f32, tag="u")
                    nc.vector.scalar_tensor_tensor(
                        u[:], SP[:], dinvS[:, w:w + 1],
                        bgB[:, l * F:(l + 1) * F], AL.mult, AL.add)
                    v = work.tile([128, F], f32, tag="v")
                    nc.vector.scalar_tensor_tensor(
                        v[:], hbuf[:, w * F:(w + 1) * F], dinvS[:, w:w + 1],
                        u[:], AL.mult, AL.add)
                    xn = work.tile([128, F], bf16, tag="xn")
                    nc.scalar.activation(xn[:], v[:], ACT.Relu)

                    if not last:
                        xtP = ps1.tile([128, F], bf16, tag="xtP")
                        nc.tensor.transpose(xtP[:], xn[:], ident[:])
                        xt = work.tile([128, F], bf16, tag="xt")
                        nc.vector.tensor_copy(xt[:], xtP[:])
                        hP = ps.tile([128, F], f32, tag="hP")
                        nc.tensor.matmul(hP[:], xt[:],
                                         wgS[:, (l + 1) * F:(l + 2) * F],
                                         start=True, stop=True)
                        nc.vector.tensor_scalar(
                            hbuf[:, w * F:(w + 1) * F], hP[:],
                            dinvS[:, w:w + 1], None, AL.mult)
                        if w == WA - 1:
                            emit_shard_half(0, (l + 1) % 2)
                    else:
                        ig = work.tile([128, GR], bf16, tag="ig")
                        nc.vector.tensor_scalar(ig[:], iotaF[:],
                                                batchS[:, w:w + 1], None,
                                                AL.is_equal)
                        nc.tensor.matmul(poolP[:], xn[:], ig[:],
                                         start=(w == 0), stop=(w == W - 1),
                                         skip_group_check=True)
                        nc.tensor.matmul(cntP[:], onesB[:], ig[:],
                                         start=(w == 0), stop=(w == W - 1),
                                         skip_group_check=True)
                assert ci == len(call_plan)
                if not last:
                    emit_shard_half(1, (l + 1) % 2)

            # ---------------- pooling allreduce + head ----------------
            sumsS = work.tile([128, GR], f32, tag="sumsS")
            nc.vector.tensor_copy(sumsS[:], poolP[:])
            cntS = work.tile([1, GR], f32, tag="cntS")
            nc.vector.tensor_copy(cntS[:], cntP[:])
            nc.sync.dma_start(arInD[0:128, :], sumsS[:])
            nc.sync.dma_start(arInD[128:129, :], cntS[:])
            nc.gpsimd.collective_compute(
                "AllReduce", mybir.AluOpType.add, replica_groups=rg,
                ins=[arInD[:]], outs=[arOutD[:]])
            sumsA = work.tile([128, GR], f32, tag="sumsA")
            nc.sync.dma_start(sumsA[:], arOutD[0:128, :])
            cntA = work.tile([1, GR], f32, tag="cntA")
            nc.sync.dma_start(cntA[:], arOutD[128:129, :])
            cntM = work.tile([1, GR], f32, tag="cntM")
            nc.vector.tensor_scalar(cntM[:], cntA[:], 1.0, None, AL.max)
            rec = work.tile([1, GR], f32, tag="rec")
            nc.vector.reciprocal(rec[:], cntM[:])
            recB = work.tile([128, GR], f32, tag="recB")
            nc.gpsimd.partition_broadcast(recB[:], rec[:])
            pooledT = work.tile([128, GR], f32, tag="pooledT")
            nc.vector.tensor_tensor(pooledT[:], sumsA[:], recB[:], AL.mult)

            h1 = []
            for h in range(2):
                h1P = ps3.tile([128, GR], f32, tag="SP")
                nc.tensor.matmul(h1P[:], w1S[:, h * 128:(h + 1) * 128],
                                 pooledT[:], start=True, stop=True)
                h1S = work.tile([128, GR], f32, tag=f"h1S{h}")
                nc.scalar.activation(h1S[:], h1P[:], ACT.Relu,
                                     bias=b1S[:, h:h + 1])
                h1.append(h1S)
            for g in range(GR // 128):
                oP = ps3.tile([128, cfg.LD], f32, tag="SP")
                nc.tensor.matmul(oP[:], h1[0][:, g * 128:(g + 1) * 128],
                                 w2S[:, 0:cfg.LD], start=True, stop=False)
                nc.tensor.matmul(oP[:], h1[1][:, g * 128:(g + 1) * 128],
                                 w2S[:, cfg.LD:2 * cfg.LD], start=False, stop=True)
                oS = work.tile([128, cfg.LD], f32, tag="oS")
                nc.vector.tensor_tensor(oS[:], oP[:], b2B[:], AL.add)
                nc.sync.dma_start(out_ext[g * 128:(g + 1) * 128, :], oS[:])

    nc.compile()
    return nc


# ---------------------------------------------------------------------------
# Entry point
# ---------------------------------------------------------------------------
_CACHE = {}


def _build(cfg, meta):
    key = (tuple(meta.m_low), tuple(meta.m_high))
    if key not in _CACHE:
        _CACHE[key] = build_graph(cfg, meta)
    return _CACHE[key]


def kernel(**inputs) -> np.ndarray:
    from concourse.bass_utils import run_bass_kernel_spmd
    cfg = Cfg()
    meta = host_prep(cfg, **inputs)
    nc = _build(cfg, meta)
    res = run_bass_kernel_spmd(nc, meta.in_maps, list(range(cfg.C)))
    return np.asarray(res.results[0]["out"], dtype=np.float32)



# revision 6
# speedup vs baseline: 2.2374x; 1.0608x over previous
"""GCN (3-layer message passing + mean-pool + MLP head) on 8 Trainium2 NeuronCores.

Sharding: nodes and their incident (by dst) edges are sharded across 8 cores;
per layer the dinv-scaled features are AllGathered into a bf16 gather table in
DRAM, each core dma_gathers its edges' src rows (4 SWDGE queues, round-robin,
-1-padded index tails so the Q7 desc-gen skips padding) and segment-sums them
with indicator matmuls on the TensorEngine. Indicator tiles are prebuilt on
the host (dinv[dst] folded in) and streamed from DRAM; deg/dinv, the pooling
one-hot (1/cnt folded) and x^T are host-side structure prep. bg enters via a
reserved table row + an all-ones indicator slot. Pooling is an indicator
matmul plus one [128,256] AllReduce; the MLP head is computed redundantly.
"""

import numpy as np
import ml_dtypes
from dataclasses import dataclass, field

BF16 = ml_dtypes.bfloat16


# ---------------------------------------------------------------------------
# Config
# ---------------------------------------------------------------------------
@dataclass
class Cfg:
    N: int = 50000          # nodes
    E: int = 600000         # edges
    F: int = 128            # feature dim
    NL: int = 3             # gcn layers
    G: int = 256            # graphs
    H: int = 256            # hidden dim of head
    LD: int = 2             # label dim
    C: int = 8              # cores
    WAW: int = 25           # windows in table half A
    PF: int = 4             # gather prefetch distance (windows)
    PFI: int = 2            # indicator-DMA prefetch distance

    @property
    def NPC(self):          # nodes per core
        return self.N // self.C

    @property
    def W(self):            # 128-node windows per core
        return (self.NPC + 127) // 128

    @property
    def NPAD(self):         # padded nodes per core
        return self.W * 128

    @property
    def TROWS(self):        # gather-table rows
        return self.C * self.NPAD

    @property
    def HA(self):           # local rows in table half A (window-aligned)
        return 128 * self.WAW

    @property
    def HB(self):           # local rows in table half B
        return self.NPAD - self.HA

    @property
    def SPLIT(self):        # low/high gather-stream boundary = half-A rows
        return self.C * self.HA


@dataclass
class Meta:
    """Uniform (core-independent) graph structure + per-core data arrays."""
    m_low: list = field(default_factory=list)    # per-window low subchunks
    m_high: list = field(default_factory=list)   # per-window high subchunks
    in_maps: list = field(default_factory=list)  # per-core tensor dicts


# ---------------------------------------------------------------------------
# Host-side sharding / layout prep (structure only, no weight math)
# ---------------------------------------------------------------------------
def _wrap16(arr_i16):
    # slot i -> [i % 16, i // 16]; 16-row wrap replicated to 128 partitions
    # (one copy per GPSIMD Q7 core).
    return np.ascontiguousarray(np.tile(arr_i16.reshape(-1, 16).T, (8, 1)))


def host_prep(cfg: Cfg, x, Wg, bg, w1, b1, w2, b2, edge_index, batch) -> Meta:
    C, NPC, W, NPAD, F, G = cfg.C, cfg.NPC, cfg.W, cfg.NPAD, cfg.F, cfg.G
    HA, HB = cfg.HA, cfg.HB
    src = np.asarray(edge_index[0], dtype=np.int64)
    dst = np.asarray(edge_index[1], dtype=np.int64)
    batch = np.asarray(batch, dtype=np.int64)
    x = np.asarray(x, dtype=np.float32)

    # GCN norm (self-loops included): structure-only prep
    deg = np.bincount(dst, minlength=cfg.N).astype(np.float64) + 1.0
    dinv = (1.0 / np.sqrt(deg)).astype(np.float32)

    # table row of a global node id: two stacked AllGather halves
    sc, sl = src // NPC, src % NPC
    trow = np.where(sl < HA, sc * HA + sl, C * HA + sc * HB + (sl - HA))
    J_BG = 0 * HB + (NPC - HA)   # core-0 pad row 6250, high-stream-relative

    # per (core, window, half) edge lists
    per_core = []
    for c in range(C):
        m = (dst // NPC) == c
        s_c, d_c, t_c = src[m], dst[m], trow[m]
        dloc = d_c - c * NPC
        order = np.argsort(dloc, kind="stable")
        dloc, t_c = dloc[order], t_c[order]
        dv = dinv[d_c[order]]
        win = dloc // 128
        drel = dloc - win * 128
        lowm = t_c < cfg.SPLIT
        lists = []
        for w in range(W):
            wm = win == w
            lists.append((
                (t_c[wm & lowm], drel[wm & lowm], dv[wm & lowm]),
                (t_c[wm & ~lowm] - cfg.SPLIT, drel[wm & ~lowm], dv[wm & ~lowm]),
            ))
        per_core.append(lists)

    # uniform subchunk counts (max over cores); high stream carries one extra
    # bg slot per window (slot 0 of the window's first high subchunk)
    m_low = [max(1, max(-(-len(per_core[c][w][0][0]) // 128) for c in range(C)))
             for w in range(W)]
    m_high = [max(1, max(-(-(len(per_core[c][w][1][0]) + 1) // 128)
                         for c in range(C)))
              for w in range(W)]
    L_low = 128 * sum(m_low)
    L_high = 128 * sum(m_high)
    S = sum(m_low) + sum(m_high)

    meta = Meta(m_low=m_low, m_high=m_high)

    # global graph-level pooling weights (1/cnt folded into the one-hot)
    cnt = np.bincount(batch, minlength=G).astype(np.float32)
    inv_cnt = 1.0 / np.maximum(cnt, 1.0)

    # per-window subchunk offsets (window-major: L subchunks then H subchunks)
    gsub0 = []
    o = 0
    for w in range(W):
        gsub0.append(o)
        o += m_low[w] + m_high[w]
    assert o == S

    wgb = np.concatenate([np.asarray(Wg[l], np.float32) for l in range(cfg.NL)],
                         axis=1).astype(BF16)                       # [F, NL*F]
    bgrow = np.asarray(bg, np.float32).reshape(1, cfg.NL * F).astype(BF16)
    identB = np.eye(128, dtype=np.float32).astype(BF16)

    for c in range(C):
        idx_low = np.zeros(L_low, np.int16)
        idx_high = np.zeros(L_high, np.int16)
        # indicator blocks [S, 128slot, 128dst] f32
        ind = np.zeros((S, 128, 128), np.float32)
        ol = oh = 0
        for w in range(W):
            (tl, dl, vl), (th, dh, vh) = per_core[c][w]
            s0 = gsub0[w]
            # low stream: edges from slot 0
            nl = len(tl)
            idx_low[ol:ol + nl] = tl.astype(np.int16)
            sub = np.arange(nl) // 128
            ind[s0 + sub, np.arange(nl) % 128, dl] = vl
            ol += 128 * m_low[w]
            # high stream: bg slot 0, edges from slot 1
            sh0 = s0 + m_low[w]
            nh = len(th)
            idx_high[oh] = np.int16(J_BG)
            idx_high[oh + 1:oh + 1 + nh] = th.astype(np.int16)
            ind[sh0, 0, :] = 1.0
            sub = (1 + np.arange(nh)) // 128
            ind[sh0 + sub, (1 + np.arange(nh)) % 128, dh] = vh
            oh += 128 * m_high[w]
        assert ol == L_low and oh == L_high
        indT = np.ascontiguousarray(
            ind.transpose(1, 0, 2).reshape(128, S * 128)).astype(BF16)

        # x^T resident [F, NPAD] bf16
        xs = np.zeros((NPAD, F), np.float32)
        xs[:NPC] = x[c * NPC:(c + 1) * NPC]
        xsT = np.ascontiguousarray(xs.T).astype(BF16)

        # dinv per local node [128, W], pads 0
        dv = np.zeros(NPAD, np.float32)
        dv[:NPC] = dinv[c * NPC:(c + 1) * NPC]
        dinvS = np.ascontiguousarray(dv.reshape(W, 128).T)

        # pooling one-hot with 1/cnt folded [128, W*G]
        ig = np.zeros((NPAD, G), np.float32)
        bloc = batch[c * NPC:(c + 1) * NPC]
        ig[np.arange(NPC), bloc] = inv_cnt[bloc]
        igS = np.ascontiguousarray(
            ig.reshape(W, 128, G).transpose(1, 0, 2).reshape(128, W * G)
        ).astype(BF16)

        b2B = np.tile(np.asarray(b2, np.float32).reshape(1, cfg.LD), (128, 1))

        meta.in_maps.append(dict(
            xsT=xsT,
            src_low=_wrap16(idx_low),
            src_high=_wrap16(idx_high),
            ind=indT,
            igS=igS,
            dinvS=dinvS,
            wgb=wgb,
            bgrow=bgrow,
            identB=identB,
            w1=np.asarray(w1, np.float32),
            b1=np.asarray(b1, np.float32).reshape(cfg.H, 1),
            w2=np.asarray(w2, np.float32),
            b2B=b2B,
        ))
    return meta


# ---------------------------------------------------------------------------
# Device graph
# ---------------------------------------------------------------------------
def build_graph(cfg: Cfg, meta: Meta):
    import concourse.bass as bass
    import concourse.bacc as bacc
    import concourse.mybir as mybir
    import concourse.tile as tile

    F, W, NL, NPAD = cfg.F, cfg.W, cfg.NL, cfg.NPAD
    GR = cfg.G
    f32, bf16, i16 = mybir.dt.float32, mybir.dt.bfloat16, mybir.dt.int16
    AL = mybir.AluOpType
    ACT = mybir.ActivationFunctionType
    mL, mH = meta.m_low, meta.m_high
    L_low, L_high = 128 * sum(mL), 128 * sum(mH)
    S = sum(mL) + sum(mH)
    MLmax, MHmax = max(mL), max(mH)
    MImax = max(mL[w] + mH[w] for w in range(W))
    WA = cfg.WAW
    HA = cfg.HA

    # per-window offsets
    offL = []
    offH = []
    offS = []
    a = b = s = 0
    for w in range(W):
        offL.append(a)
        offH.append(b)
        offS.append(s)
        a += 128 * mL[w]
        b += 128 * mH[w]
        s += mL[w] + mH[w]

    nc = bacc.Bacc("TRN2", target_bir_lowering=False, debug=False,
                   num_devices=cfg.C, num_swdge_queues=4)

    # --- external IO ------------------------------------------------------
    P = {}
    P["xsT"] = nc.declare_dram_parameter("xsT", [F, NPAD], bf16, isOutput=False)
    P["src_low"] = nc.declare_dram_parameter("src_low", [128, L_low // 16], i16, isOutput=False)
    P["src_high"] = nc.declare_dram_parameter("src_high", [128, L_high // 16], i16, isOutput=False)
    P["ind"] = nc.declare_dram_parameter("ind", [128, S * 128], bf16, isOutput=False)
    P["igS"] = nc.declare_dram_parameter("igS", [128, W * GR], bf16, isOutput=False)
    P["dinvS"] = nc.declare_dram_parameter("dinvS", [128, W], f32, isOutput=False)
    P["wgb"] = nc.declare_dram_parameter("wgb", [F, NL * F], bf16, isOutput=False)
    P["bgrow"] = nc.declare_dram_parameter("bgrow", [1, NL * F], bf16, isOutput=False)
    P["identB"] = nc.declare_dram_parameter("identB", [128, 128], bf16, isOutput=False)
    P["w1"] = nc.declare_dram_parameter("w1", [F, cfg.H], f32, isOutput=False)
    P["b1"] = nc.declare_dram_parameter("b1", [cfg.H, 1], f32, isOutput=False)
    P["w2"] = nc.declare_dram_parameter("w2", [cfg.H, cfg.LD], f32, isOutput=False)
    P["b2B"] = nc.declare_dram_parameter("b2B", [128, cfg.LD], f32, isOutput=False)
    out_ext = nc.declare_dram_parameter("out", [GR, cfg.LD], f32, isOutput=True)

    # --- internal DRAM ----------------------------------------------------
    tableDs = [nc.dram_tensor(f"tableD{i}", [cfg.TROWS, F], bf16,
                              addr_space="Shared") for i in range(2)]
    shardDs = [nc.dram_tensor(f"shardD{i}", [NPAD, F], bf16) for i in range(2)]
    arInD = nc.dram_tensor("arInD", [128, GR], f32)
    arOutD = nc.dram_tensor("arOutD", [128, GR], f32, addr_space="Shared")

    rg = [list(range(cfg.C))]

    with tile.TileContext(nc) as tc:
        with (
            tc.tile_pool(name="res", bufs=1) as res,      # resident tensors
            tc.tile_pool(name="work", bufs=4) as work,    # per-window temps
            tc.tile_pool(name="indp", bufs=4) as indp,    # indicator stream
            tc.tile_pool(name="gl", bufs=6) as glp,       # low gather bufs
            tc.tile_pool(name="gh", bufs=6) as ghp,       # high gather bufs
            tc.tile_pool(name="ps", bufs=2, space="PSUM") as ps,
            tc.tile_pool(name="ps3", bufs=4, space="PSUM") as ps3,
            tc.tile_pool(name="ps1", bufs=1, space="PSUM") as ps1,
            tc.tile_pool(name="psacc", bufs=1, space="PSUM") as psacc,
        ):
            # ---------------- resident loads ----------------
            srcL = res.tile([128, L_low // 16], i16)
            nc.sync.dma_start(srcL[:], P["src_low"][:])
            srcH = res.tile([128, L_high // 16], i16)
            nc.sync.dma_start(srcH[:], P["src_high"][:])
            dinvS = res.tile([128, W], f32)
            nc.sync.dma_start(dinvS[:], P["dinvS"][:])
            xsT = res.tile([128, NPAD], bf16)
            nc.sync.dma_start(xsT[:], P["xsT"][:])
            igS = res.tile([128, W * GR], bf16)
            nc.sync.dma_start(igS[:], P["igS"][:])
            wgS = res.tile([128, NL * F], bf16)
            nc.sync.dma_start(wgS[:], P["wgb"][:])
            ident = res.tile([128, 128], bf16)
            nc.sync.dma_start(ident[:], P["identB"][:])
            w1S = res.tile([128, cfg.H], f32)
            nc.sync.dma_start(w1S[:], P["w1"][:])
            w2S = res.tile([128, 2 * cfg.LD], f32)
            nc.sync.dma_start(w2S[:, 0:cfg.LD], P["w2"][0:128, :])
            nc.sync.dma_start(w2S[:, cfg.LD:2 * cfg.LD], P["w2"][128:256, :])
            b1S = res.tile([128, 2], f32)
            nc.sync.dma_start(b1S[:, 0:1], P["b1"][0:128, :])
            nc.sync.dma_start(b1S[:, 1:2], P["b1"][128:256, :])
            b2B = res.tile([128, cfg.LD], f32)
            nc.sync.dma_start(b2B[:], P["b2B"][:])

            hbuf = res.tile([128, W * F], bf16)      # h' chunks, node-major

            # init gather pools (stale-SBUF NaN guard for trimmed tails)
            ztiles = []
            for _ in range(6):
                t = glp.tile([128, MLmax * F], bf16, tag="gL")
                nc.vector.memset(t[:], 0.0)
                ztiles.append(t)
                t = ghp.tile([128, MHmax * F], bf16, tag="gH")
                nc.vector.memset(t[:], 0.0)
                ztiles.append(t)

            def reg_of(n):
                return n

            def emit_shard_half(half, par):
                shardD, tableD = shardDs[par], tableDs[par]
                r0, r1 = (0, HA) if half == 0 else (HA, NPAD)
                t0 = 0 if half == 0 else cfg.C * HA
                nc.sync.dma_start(
                    shardD[r0:r1].rearrange("(w p) f -> p w f", p=128),
                    hbuf[:, r0 * F // 128:r1 * F // 128]
                    .rearrange("p (w f) -> p w f", f=F))
                nc.gpsimd.collective_compute(
                    "AllGather", mybir.AluOpType.bypass, replica_groups=rg,
                    ins=[shardD[r0:r1]],
                    outs=[tableD[t0:t0 + cfg.C * (r1 - r0)]])

            def poke_bg(l):
                # write bg_l into hbuf pad row (node 6250: window 48, p=106)
                pw = cfg.NPC // 128
                pp = cfg.NPC - pw * 128
                nc.sync.dma_start(hbuf[pp:pp + 1, pw * F:(pw + 1) * F],
                                  P["bgrow"][:, l * F:(l + 1) * F])

            # ---------------- phase A: h'_0 = dinv * (x @ Wg0) -----------
            for w in range(W):
                hP = ps.tile([128, F], f32, tag="hP")
                nc.tensor.matmul(hP[:], xsT[:, w * 128:(w + 1) * 128],
                                 wgS[:, 0:F], start=True, stop=True)
                nc.scalar.activation(hbuf[:, w * F:(w + 1) * F], hP[:],
                                     ACT.Copy, scale=dinvS[:, w:w + 1])
                if w == WA - 1:
                    emit_shard_half(0, 0)
            poke_bg(0)
            emit_shard_half(1, 0)

            # ---------------- layers ----------------
            qctr = [0]
            poolP = None
            for l in range(NL):
                tableD = tableDs[l % 2]
                last = l == NL - 1
                if last:
                    poolP = psacc.tile([128, GR], f32, tag="poolP")

                gtilesL = {}
                gtilesH = {}

                def emit_window_gathers(w):
                    nL = 128 * mL[w]
                    gt = glp.tile([128, MLmax * F], bf16, tag="gL")
                    nc.gpsimd.dma_gather(
                        gt[:, :mL[w] * F].rearrange("p (c e) -> p c e", e=F),
                        tableD[0:cfg.SPLIT, :],
                        srcL[:, offL[w] // 16:(offL[w] + nL) // 16],
                        nL, reg_of(nL), F,
                        queue_num=qctr[0] % 4)
                    qctr[0] += 1
                    gtilesL[w] = gt
                    nH = 128 * mH[w]
                    gt = ghp.tile([128, MHmax * F], bf16, tag="gH")
                    nc.gpsimd.dma_gather(
                        gt[:, :mH[w] * F].rearrange("p (c e) -> p c e", e=F),
                        tableD[cfg.SPLIT:cfg.TROWS, :],
                        srcH[:, offH[w] // 16:(offH[w] + nH) // 16],
                        nH, reg_of(nH), F,
                        queue_num=qctr[0] % 4)
                    qctr[0] += 1
                    gtilesH[w] = gt

                indtiles = {}

                def emit_window_ind(w):
                    nsub = mL[w] + mH[w]
                    it = indp.tile([128, MImax * 128], bf16, tag="ind")
                    nc.sync.dma_start(
                        it[:, :nsub * 128],
                        P["ind"][:, offS[w] * 128:(offS[w] + nsub) * 128])
                    indtiles[w] = it

                ew = ei = 0
                for w in range(W):
                    while ew <= min(w + cfg.PF, W - 1):
                        emit_window_gathers(ew)
                        ew += 1
                    while ei <= min(w + cfg.PFI, W - 1):
                        emit_window_ind(ei)
                        ei += 1

                    nsub = mL[w] + mH[w]
                    it = indtiles.pop(w)
                    gtl = gtilesL.pop(w)
                    gth = gtilesH.pop(w)
                    SP = ps3.tile([128, F], f32, tag="SP")
                    for i in range(nsub):
                        if i < mL[w]:
                            gsl = gtl[:, i * F:(i + 1) * F]
                        else:
                            j = i - mL[w]
                            gsl = gth[:, j * F:(j + 1) * F]
                        nc.tensor.matmul(SP[:],
                                         it[:, i * 128:(i + 1) * 128],
                                         gsl,
                                         start=(i == 0), stop=(i == nsub - 1))
                    # v = hbuf*dinv + SP ; xn = relu(v)
                    v = work.tile([128, F], f32, tag="v")
                    nc.vector.scalar_tensor_tensor(
                        v[:], hbuf[:, w * F:(w + 1) * F], dinvS[:, w:w + 1],
                        SP[:], AL.mult, AL.add)
                    xn = work.tile([128, F], bf16, tag="xn")
                    nc.scalar.activation(xn[:], v[:], ACT.Relu)

                    if not last:
                        xtP = ps1.tile([128, F], bf16, tag="xtP")
                        nc.tensor.transpose(xtP[:], xn[:], ident[:])
                        xt = work.tile([128, F], bf16, tag="xt")
                        nc.vector.tensor_copy(xt[:], xtP[:])
                        hP = ps.tile([128, F], f32, tag="hP")
                        nc.tensor.matmul(hP[:], xt[:],
                                         wgS[:, (l + 1) * F:(l + 2) * F],
                                         start=True, stop=True)
                        nc.scalar.activation(hbuf[:, w * F:(w + 1) * F], hP[:],
                                             ACT.Copy, scale=dinvS[:, w:w + 1])
                        if w == WA - 1:
                            emit_shard_half(0, (l + 1) % 2)
                        if w == W - 1:
                            poke_bg(l + 1)
                            emit_shard_half(1, (l + 1) % 2)
                    else:
                        nc.tensor.matmul(poolP[:], xn[:],
                                         igS[:, w * GR:(w + 1) * GR],
                                         start=(w == 0), stop=(w == W - 1),
                                         skip_group_check=True)
                assert ew == W and ei == W

            # ---------------- pooling allreduce + head ----------------
            sumsS = work.tile([128, GR], f32, tag="sumsS")
            nc.vector.tensor_copy(sumsS[:], poolP[:])
            nc.sync.dma_start(arInD[:], sumsS[:])
            nc.gpsimd.collective_compute(
                "AllReduce", mybir.AluOpType.add, replica_groups=rg,
                ins=[arInD[:]], outs=[arOutD[:]])
            pooledT = work.tile([128, GR], f32, tag="pooledT")
            nc.sync.dma_start(pooledT[:], arOutD[:])

            h1 = []
            for h in range(2):
                h1P = ps3.tile([128, GR], f32, tag="SP")
                nc.tensor.matmul(h1P[:], w1S[:, h * 128:(h + 1) * 128],
                                 pooledT[:], start=True, stop=True)
                h1S = work.tile([128, GR], f32, tag=f"h1S{h}")
                nc.scalar.activation(h1S[:], h1P[:], ACT.Relu,
                                     bias=b1S[:, h:h + 1])
                h1.append(h1S)
            for g in range(GR // 128):
                oP = ps3.tile([128, cfg.LD], f32, tag="SP")
                nc.tensor.matmul(oP[:], h1[0][:, g * 128:(g + 1) * 128],
                                 w2S[:, 0:cfg.LD], start=True, stop=False)
                nc.tensor.matmul(oP[:], h1[1][:, g * 128:(g + 1) * 128],
                                 w2S[:, cfg.LD:2 * cfg.LD], start=False, stop=True)
                oS = work.tile([128, cfg.LD], f32, tag="oS")
                nc.vector.tensor_tensor(oS[:], oP[:], b2B[:], AL.add)
                nc.sync.dma_start(out_ext[g * 128:(g + 1) * 128, :], oS[:])

    nc.compile()
    return nc


# ---------------------------------------------------------------------------
# Entry point
# ---------------------------------------------------------------------------
_CACHE = {}


def _build(cfg, meta):
    key = (tuple(meta.m_low), tuple(meta.m_high))
    if key not in _CACHE:
        _CACHE[key] = build_graph(cfg, meta)
    return _CACHE[key]


def kernel(**inputs) -> np.ndarray:
    from concourse.bass_utils import run_bass_kernel_spmd
    cfg = Cfg()
    meta = host_prep(cfg, **inputs)
    nc = _build(cfg, meta)
    res = run_bass_kernel_spmd(nc, meta.in_maps, list(range(cfg.C)))
    return np.asarray(res.results[0]["out"], dtype=np.float32)
